# revision 4
# baseline (speedup 1.0000x reference)
"""Bi-directional WKV kernel for Trainium2, 8-core batch-parallel. V2.

Measured-cost-driven engine split (DVE scan = 2.11 ns/elem is the floor):
  - DVE: 8 chained scans (4 streams x 2 PSUM banks), ekv mult (bf16 2x),
    num join (bf16 2x), X mult (bf16 2x), reciprocal, output merge stt.
  - ACT (one pinned table exp_and_others: Exp/Tanh/Copy): ekN=e^k,
    ekD=e^(k+lns), v->bf16, th=tanh(r/2), c1-scaled PSUM->SBUF copies,
    rd->bf16.
  - Pool: the three remaining big adds (t1=nf+nb, t2=df+db, den=t2+cek).
  - PE: all transposes in bf16 (1 cyc/row), 1-bank PSUM tiles.

Scaling trick: ekD carries s=2/scale_b so rd=recip(den') = (scale_b/2)/den;
out = (th+1)*X^T with X=num*rd gives sigmoid(r)*scale_b*num/den exactly.
Host precomputes decay/c1/lns per batch; device needs no Sigmoid/Ln table.
"""

import numpy as np
from contextlib import ExitStack

import concourse.bass as bass
import concourse.bacc as bacc
import concourse.tile as tile
from concourse import mybir
from concourse.bass_utils import run_bass_kernel_spmd
from concourse.masks import make_identity

from concourse.hw_specs import get_activation_tables


def _pin_act_tables():
    tabs = get_activation_tables("gen3")
    keep = {"exp_and_others"}
    for name in list(tabs):
        if name not in keep:
            tabs[name] = set()


_pin_act_tables()

B, T, C, TD = 8, 2048, 2048, 512
P = 128
N_CORES = 8
f32 = mybir.dt.float32
bf16 = mybir.dt.bfloat16
ALU = mybir.AluOpType
AF = mybir.ActivationFunctionType

CC = 256          # channels per slab (2 ctiles)
NSLAB = C // CC
NT = T // P       # 16 t-blocks
HB = 1024         # psum bank width in bf16 elems


def _body(tc, out, r, k, v, decay, c1, lns):
    nc = tc.nc
    NCT = C // P

    with ExitStack() as ctx:
        consts = ctx.enter_context(tc.tile_pool(name="consts", bufs=1))
        slabs = ctx.enter_context(tc.tile_pool(name="slabs", bufs=2))
        work = ctx.enter_context(tc.tile_pool(name="work", bufs=1))
        psum = ctx.enter_context(tc.tile_pool(name="psum", bufs=1, space="PSUM"))

        ident = consts.tile([P, P], bf16)
        make_identity(nc, ident[:])

        dect = consts.tile([P, NCT], f32)
        nc.gpsimd.dma_start(out=dect[:], in_=decay.rearrange("(j p) -> p j", p=P))
        c1t = consts.tile([P, NCT], f32)
        nc.gpsimd.dma_start(out=c1t[:], in_=c1.rearrange("(j p) -> p j", p=P))
        lnst = consts.tile([P, 1], f32)
        lns_b = bass.AP(tensor=lns.tensor, offset=lns.offset,
                        ap=[[0, P]] + list(lns.ap))
        nc.gpsimd.dma_start(out=lnst[:], in_=lns_b)

        def slab_src(ap, s):
            return ap.rearrange("(tc tp) (s cc) -> tp tc s cc", tp=P, cc=CC)[:, :, s, :]

        for s in range(NSLAB):
            kslab = slabs.tile([P, NT, CC], f32, tag="kslab")
            vslab = slabs.tile([P, NT, CC], f32, tag="vslab")
            rslab = slabs.tile([P, NT, CC], f32, tag="rslab")
            nc.sync.dma_start(out=kslab[:], in_=slab_src(k, s))
            nc.sync.dma_start(out=vslab[:], in_=slab_src(v, s))
            nc.sync.dma_start(out=rslab[:], in_=slab_src(r, s))
            oslab = slabs.tile([P, NT, CC], f32, tag="oslab")

            # bf16 views inside the f32 v slab: lower half = vb, upper = ekv
            vb = vslab[:].bitcast(bf16)          # [P, NT, 2*CC] bf16 view
            ekvv = vb[:, :, CC:2 * CC]
            vbv = vb[:, :, 0:CC]

            ekD = slabs.tile([P, NT, CC], bf16, tag="ekD")   # e^(k+lns)
            ekN = work.tile([P, NT, CC], bf16, tag="ekN")    # e^k
            nc.scalar.activation(out=ekD[:], in_=kslab[:], func=AF.Exp,
                                 bias=lnst[:, 0:1])
            nc.scalar.activation(out=ekN[:], in_=kslab[:], func=AF.Exp)
            nc.scalar.activation(out=vbv, in_=vslab[:], func=AF.Copy)
            # th = tanh(r/2) in place (f32)
            nc.scalar.activation(out=rslab[:], in_=rslab[:], func=AF.Tanh, scale=0.5)
            # ekv = ekN * vb on the otherwise-idle Pool engine; slab 0 uses
            # DVE (idle during pipeline fill) to shorten the startup chain
            ekv_eng = nc.vector if s == 0 else nc.gpsimd
            ekv_eng.tensor_tensor(ekvv, ekN[:], vbv, ALU.mult)

            for half in range(2):
                j = 2 * s + half
                co = half * P
                dj = dect[:, j:j + 1]
                djb0 = dj.broadcast_to((P, HB))
                cj = c1t[:, j:j + 1]

                pv0 = psum.tile([P, HB], bf16, tag="pv0")
                pv1 = psum.tile([P, HB], bf16, tag="pv1")
                pk0 = psum.tile([P, HB], bf16, tag="pk0")
                pk1 = psum.tile([P, HB], bf16, tag="pk1")
                for t_ in range(NT):
                    bs = slice((t_ % 8) * P, (t_ % 8 + 1) * P)
                    pvt = pv0 if t_ < 8 else pv1
                    pkt = pk0 if t_ < 8 else pk1
                    nc.tensor.transpose(pvt[:, bs], ekvv[:, t_, co:co + P], ident[:])
                    nc.tensor.transpose(pkt[:, bs], ekD[:, t_, co:co + P], ident[:])

                nf = work.tile([P, T], bf16, tag="nf", bufs=2)
                df = work.tile([P, T], bf16, tag="df", bufs=2)
                nb = work.tile([P, T], bf16, tag="nb", bufs=2)
                db = work.tile([P, T], bf16, tag="db", bufs=2)
                # forward scans (chained across the two banks)
                nc.vector.tensor_tensor_scan(out=nf[:, 0:HB], data0=djb0,
                                             data1=pv0[:], initial=0.0,
                                             op0=ALU.mult, op1=ALU.add)
                nc.vector.tensor_tensor_scan(out=nf[:, HB:T], data0=djb0,
                                             data1=pv1[:],
                                             initial=nf[:, HB - 1:HB],
                                             op0=ALU.mult, op1=ALU.add)
                nc.vector.tensor_tensor_scan(out=df[:, 0:HB], data0=djb0,
                                             data1=pk0[:], initial=0.0,
                                             op0=ALU.mult, op1=ALU.add)
                nc.vector.tensor_tensor_scan(out=df[:, HB:T], data0=djb0,
                                             data1=pk1[:],
                                             initial=df[:, HB - 1:HB],
                                             op0=ALU.mult, op1=ALU.add)
                # backward scans (reversed views, chained)
                nc.vector.tensor_tensor_scan(out=nb[:, T - 1:HB - 1:-1],
                                             data0=djb0,
                                             data1=pv1[:, HB - 1::-1],
                                             initial=0.0,
                                             op0=ALU.mult, op1=ALU.add)
                nc.vector.tensor_tensor_scan(out=nb[:, HB - 1::-1], data0=djb0,
                                             data1=pv0[:, HB - 1::-1],
                                             initial=nb[:, HB:HB + 1],
                                             op0=ALU.mult, op1=ALU.add)
                nc.vector.tensor_tensor_scan(out=db[:, T - 1:HB - 1:-1],
                                             data0=djb0,
                                             data1=pk1[:, HB - 1::-1],
                                             initial=0.0,
                                             op0=ALU.mult, op1=ALU.add)
                nc.vector.tensor_tensor_scan(out=db[:, HB - 1::-1], data0=djb0,
                                             data1=pk0[:, HB - 1::-1],
                                             initial=db[:, HB:HB + 1],
                                             op0=ALU.mult, op1=ALU.add)

                # c1-scaled copies of the PSUM streams
                cekv = work.tile([P, T], bf16, tag="cekv")
                nc.scalar.activation(out=cekv[:, 0:HB], in_=pv0[:], func=AF.Copy,
                                     scale=cj)
                nc.scalar.activation(out=cekv[:, HB:T], in_=pv1[:], func=AF.Copy,
                                     scale=cj)
                cek = work.tile([P, T], bf16, tag="cek")
                nc.scalar.activation(out=cek[:, 0:HB], in_=pk0[:], func=AF.Copy,
                                     scale=cj)
                nc.scalar.activation(out=cek[:, HB:T], in_=pk1[:], func=AF.Copy,
                                     scale=cj)

                # t1 = nf+nb (Pool, in place nb); num = t1+cekv (DVE 2x, in nf)
                nc.gpsimd.tensor_tensor(nb[:], nf[:], nb[:], ALU.add)
                nc.vector.tensor_tensor(nf[:], nb[:], cekv[:], ALU.add)
                # t2 = df+db (Pool, in db); den = t2+cek (Pool, f32 out)
                nc.gpsimd.tensor_tensor(db[:], df[:], db[:], ALU.add)
                den = work.tile([P, T], f32, tag="den")
                nc.gpsimd.tensor_tensor(den[:], db[:], cek[:], ALU.add)

                nc.vector.reciprocal_approx_fast(out=den[:], in_=den[:])
                rdb = work.tile([P, T], bf16, tag="rdb")
                nc.scalar.activation(out=rdb[:], in_=den[:], func=AF.Copy)
                # X = num * rd (bf16 2x, in nf)
                nc.vector.tensor_tensor(nf[:], nf[:], rdb[:], ALU.mult)

                px0 = psum.tile([P, 8, P], bf16, tag="px0", bufs=2)
                px1 = psum.tile([P, 8, P], bf16, tag="px1", bufs=2)
                for t_ in range(NT):
                    pxt = px0 if t_ < 8 else px1
                    nc.tensor.transpose(pxt[:, t_ % 8, :],
                                        nf[:, t_ * P:(t_ + 1) * P], ident[:])
                # out = (th + 1) * X^T
                nc.vector.scalar_tensor_tensor(
                    out=oslab[:, 0:8, co:co + P], in0=rslab[:, 0:8, co:co + P],
                    scalar=1.0, in1=px0[:], op0=ALU.add, op1=ALU.mult)
                nc.vector.scalar_tensor_tensor(
                    out=oslab[:, 8:NT, co:co + P], in0=rslab[:, 8:NT, co:co + P],
                    scalar=1.0, in1=px1[:], op0=ALU.add, op1=ALU.mult)

            nc.sync.dma_start(out=slab_src(out, s), in_=oslab[:])


def build_module():
    nc = bacc.Bacc("TRN2", target_bir_lowering=False, debug=False)
    r = nc.dram_tensor("r", [T, C], f32, kind="ExternalInput").ap()
    k = nc.dram_tensor("k", [T, C], f32, kind="ExternalInput").ap()
    v = nc.dram_tensor("v", [T, C], f32, kind="ExternalInput").ap()
    decay = nc.dram_tensor("decay", [C], f32, kind="ExternalInput").ap()
    c1 = nc.dram_tensor("c1", [C], f32, kind="ExternalInput").ap()
    lns = nc.dram_tensor("lns", [1], f32, kind="ExternalInput").ap()
    out = nc.dram_tensor("out", [T, C], f32, kind="ExternalOutput").ap()
    with tile.TileContext(nc) as tc:
        _body(tc, out, r, k, v, decay, c1, lns)
    nc.compile()
    return nc


_nc_cache = None


def run_full(r, k, v, w, u, time_emb, trace=False, **spmd_kwargs):
    global _nc_cache
    if _nc_cache is None:
        _nc_cache = build_module()
    nc = _nc_cache
    r = np.asarray(r, dtype=np.float32)
    k = np.asarray(k, dtype=np.float32)
    v = np.asarray(v, dtype=np.float32)
    w = np.asarray(w, dtype=np.float64)
    u = np.asarray(u, dtype=np.float64)
    time_emb = np.asarray(time_emb, dtype=np.float64)

    tf = 1.0 / (1.0 + np.exp(-time_emb.sum(axis=-1)))
    scale_b = 0.8 + 0.2 * tf
    dec0 = np.exp(-np.exp(w))
    c1v = np.exp(u) - 1.0
    in_maps = []
    for b in range(B):
        in_maps.append({
            "r": np.ascontiguousarray(r[b]),
            "k": np.ascontiguousarray(k[b]),
            "v": np.ascontiguousarray(v[b]),
            "decay": (dec0 * (0.5 + 0.5 * tf[b])).astype(np.float32),
            "c1": c1v.astype(np.float32),
            "lns": np.array([np.log(2.0 / scale_b[b])], dtype=np.float32),
        })
    res = run_bass_kernel_spmd(nc, in_maps, core_ids=list(range(N_CORES)),
                               trace=trace, **spmd_kwargs)
    out = np.stack([res.results[b]["out"] for b in range(B)], axis=0)
    return out, res


def kernel(r, k, v, w, u, time_emb, **extra):
    out, _ = run_full(r, k, v, w, u, time_emb)
    return out
